# revision 117
# baseline (speedup 1.0000x reference)
"""Trainium2 Bass kernel for nn_LinearAttention_40544491274679.

Computation: token embedding gather -> L=2 layers of
  [3x causal-conv FFN ladders (F->I, I->I k=3, I->F), feature-dim cumsum,
   position-normalized cell + momentum coupling] ->
1x1 conv to logits -> log_softmax -> mean NLL (scalar).

Sharding: data-parallel over (batch, sequence-quarter) = 8 shards, one per
NeuronCore. Each core processes 512 output positions with a 4-column left
halo. Weights are replicated, streamed from HBM in fp8-e4m3 (scaled by 64).

Perf design (vs the bf16 v1 baseline):
- All conv matmuls run fp8 DoubleRow (contraction pairs of 128-chunks),
  512-wide main tile + tiny halo tile.
- The positional-embedding add (h = b + fe) is folded away: conv0 streams
  b directly (fp8 x256) and the constant W0@fe enters as a per-channel bias
  column (host-precomputed in fp32) during the ReLU evacuation. A per-core
  zeroed halo-bias variant replaces all causal-mask instructions.
- Everything element-wise is bf16 (DVE 2x/4x modes); the momentum-coupling
  chain runs on the otherwise-idle GpSimd engine.
- Norm uses g = 1/sqrt(v/F + 1e-9) so all-zero halo columns stay exactly 0.
"""

import math
from contextlib import ExitStack

import numpy as np
import ml_dtypes

import concourse.bass as bass
import concourse.tile as tile
from concourse import bacc, mybir
from concourse import bass_utils

# Problem constants (hardcoded; kernel.py must be self-contained).
B, S, F, I, KW, L, C = 2, 2048, 512, 1024, 3, 2, 256
BETA = 0.99
INIT_SCALE = L ** -0.5
NCORES = 8
CHUNK = 512          # output positions per core
HALO = 4             # left halo columns
W = CHUNK + HALO     # working width per core
PITCH = 528          # fp8 tile column pitch (multiple of 16 for DoubleRow)
PAD_IDX = 300.0      # sentinel index -> all-zero one-hot -> x = 0 (causal pad)

S_W = 64.0           # weight fp8 scale (w0/w1/w2)
S_B = 256.0          # stream (b) fp8 scale
S_E = 1024.0         # embedding fp8 scale
S_W2C = 16.0         # cumsum-fused conv2(j=0) weight scale
MAIN = slice(HALO, W)

dt = mybir.dt
AF = mybir.ActivationFunctionType
OP = mybir.AluOpType
DR = mybir.MatmulPerfMode.DoubleRow

TRACE = False
_CACHE = {}


def _bcast_ap(dram_handle, n):
    """AP that reads a 1-D DRAM row replicated across 128 partitions."""
    a = dram_handle[:]
    return bass.AP(tensor=a.tensor, offset=a.offset,
                   ap=[[0, 128]] + [list(x) for x in a.ap])


def _build():
    nc = bacc.Bacc("TRN2", target_bir_lowering=False, debug=False,
                   num_devices=NCORES)

    # ---- DRAM I/O ----
    d_emb = nc.dram_tensor("emb8", [128, 2, 8, 128], dt.float8e4,
                           kind="ExternalInput")
    d_ow = nc.dram_tensor("owT", [128, 8, 2, 128], dt.bfloat16,
                          kind="ExternalInput")
    d_ob = nc.dram_tensor("obrow", [1, 256], dt.bfloat16, kind="ExternalInput")
    d_w0fe = nc.dram_tensor("w0fe", [128, 6, 8], dt.float32,
                            kind="ExternalInput")
    d_w0feh = nc.dram_tensor("w0feh", [128, 6, 8], dt.float32,
                             kind="ExternalInput")
    d_w0, d_w1, d_w2 = {}, {}, {}
    for l in range(L):
        for j in range(3):
            d_w0[(l, j)] = nc.dram_tensor(f"w0_{l}{j}", [128, 8, 4, 128],
                                          dt.float8e4, kind="ExternalInput")
            d_w1[(l, j)] = nc.dram_tensor(f"w1_{l}{j}", [128, 8, 24, 128],
                                          dt.float8e4, kind="ExternalInput")
            d_w2[(l, j)] = nc.dram_tensor(f"w2_{l}{j}", [128, 4, 8, 128],
                                          dt.float8e4, kind="ExternalInput")
    d_inp = nc.dram_tensor("inprow", [W], dt.float32, kind="ExternalInput")
    d_tgt = nc.dram_tensor("tgtrow", [CHUNK], dt.float32, kind="ExternalInput")
    d_idv = nc.dram_tensor("idvrow", [W], dt.float32, kind="ExternalInput")
    d_nll = nc.dram_tensor("nll", [1, 2 * CHUNK], dt.float32,
                           kind="ExternalOutput")

    with tile.TileContext(nc) as tc, ExitStack() as ctx:
        ctx.enter_context(nc.allow_low_precision(
            reason="final scalar NLL tolerance 2e-2; fp8/bf16 by design"))
        sb = ctx.enter_context(tc.tile_pool(name="sb", bufs=1))
        ps = ctx.enter_context(tc.tile_pool(name="ps", bufs=1,
                                            space=bass.MemorySpace.PSUM))

        def pc_tile(w=CHUNK):
            return ps.tile([128, w], dt.float32, tag="pc", bufs=6, name="pc")

        def pb_tile(w=CHUNK):
            # conv halo accumulators live in the stats banks (idle mid-layer)
            return ps.tile([128, w], dt.float32, tag="pr", bufs=2, name="pb")

        def pr_tile(w=CHUNK):
            return ps.tile([1, w], dt.float32, tag="pr", bufs=2, name="pr")

        # ---- DMA order: pipeline-gating transfers first ----
        inb = sb.tile([128, W], dt.float32, tag="ibc", name="inb")
        nc.sync.dma_start(out=inb[:], in_=_bcast_ap(d_inp, W))
        emb_sb = sb.tile([128, 2, 8, 128], dt.float8e4, tag="emb", name="emb_sb")
        nc.sync.dma_start(out=emb_sb[:], in_=d_emb[:])
        w0fe_sb = sb.tile([128, 6, 8], dt.float32, tag="w0fe", name="w0fe_sb")
        nc.sync.dma_start(out=w0fe_sb[:], in_=d_w0fe[:])
        w0feh_sb = sb.tile([128, 6, 8], dt.float32, tag="w0feh", name="w0feh_sb")
        nc.sync.dma_start(out=w0feh_sb[:], in_=d_w0feh[:])
        # idb carries 1/(pos+1) / S_W2C (scale folded on host); needed by the
        # first FFN's cum evacuation, so it loads before the weight prefetch
        idb = sb.tile([128, W], dt.float32, tag="idb", name="idb")
        nc.sync.dma_start(out=idb[:], in_=_bcast_ap(d_idv, W))

        # first-FFN weights next (w1 in halves so conv1 can start early);
        # prefetch the second FFN too before the misc constants
        w_tiles = {}
        for pf in ((0, 0),):
            w0t0 = sb.tile([128, 8, 4, 128], dt.float8e4, tag="w0", bufs=2,
                           name="w0t")
            nc.sync.dma_start(out=w0t0[:], in_=d_w0[pf][:])
            w1t0 = sb.tile([128, 8, 24, 128], dt.float8e4, tag="w1", bufs=2,
                           name="w1t")
            nc.sync.dma_start(out=w1t0[:, 0:4], in_=d_w1[pf][:, 0:4])
            nc.sync.dma_start(out=w1t0[:, 4:8], in_=d_w1[pf][:, 4:8])
            w2t0 = sb.tile([128, 4, 8, 128], dt.float8e4, tag="w2", bufs=2,
                           name="w2t")
            nc.sync.dma_start(out=w2t0[:], in_=d_w2[pf][:])
            w_tiles[pf] = (w0t0, w1t0, w2t0)

        # ---- remaining constants ----
        ow_sb = sb.tile([128, 8, 2, 128], dt.bfloat16, tag="ow", name="ow_sb")
        nc.sync.dma_start(out=ow_sb[:], in_=d_ow[:])
        ob_sb = sb.tile([1, 256], dt.bfloat16, tag="ob", name="ob_sb")
        nc.sync.dma_start(out=ob_sb[:], in_=d_ob[:])
        tgb = sb.tile([128, CHUNK], dt.float32, tag="tgb", name="tgb")
        nc.sync.dma_start(out=tgb[:], in_=_bcast_ap(d_tgt, CHUNK))

        ones_cb = sb.tile([128, 1], dt.bfloat16, tag="ocb", name="ones_cb")
        nc.vector.memset(ones_cb[:], 1.0)
        ones_row = sb.tile([1, CHUNK], dt.bfloat16, tag="orow", name="ones_row")
        nc.vector.memset(ones_row[:], 1.0)
        g_scale = float((np.float32(1.0) - np.float32(BETA)) * np.float32(INIT_SCALE))
        bcmean = sb.tile([1, 128], dt.bfloat16, tag="bcm", name="bcmean")
        nc.vector.memset(bcmean[:], 1.0 / F)
        bcg = sb.tile([1, 128], dt.bfloat16, tag="bcg", name="bcg")
        nc.vector.memset(bcg[:], g_scale)

        eps_t = sb.tile([1, 1], dt.float32, tag="eps", name="eps_t")
        nc.vector.memset(eps_t[:], 1e-9)
        # warm every activation-function table off the critical path
        scr = sb.tile([1, 1], dt.float32, tag="scr", name="scr")
        for fn_ in (AF.Relu, AF.Abs_reciprocal_sqrt):
            nc.scalar.activation(scr[:], eps_t[:], fn_)

        iota_i = sb.tile([128, 1], dt.int32, tag="ioi", name="iota_i")
        nc.gpsimd.iota(iota_i[:], [[0, 1]], base=0, channel_multiplier=1)
        iota_f = []
        for ck in range(2):
            t = sb.tile([128, 1], dt.float32, tag=f"iof{ck}", name=f"iota_f{ck}")
            if ck == 0:
                nc.vector.tensor_copy(t[:], iota_i[:])
            else:
                nc.vector.tensor_scalar_add(t[:], iota_f[0][:], 128.0)
            iota_f.append(t)

        # ---- one-hots (input early; target early, used at the end) ----
        oh = sb.tile([128, 2, PITCH], dt.float8e4, tag="oh", name="oh")
        oht = sb.tile([128, 2, CHUNK], dt.bfloat16, tag="oht", name="oht")
        for ck in range(2):
            nc.vector.tensor_scalar(oh[:, ck, 0:W], inb[:], iota_f[ck][:], None,
                                    op0=OP.is_equal)
        for ck in range(2):
            nc.vector.tensor_scalar(oht[:, ck, :], tgb[:], iota_f[ck][:], None,
                                    op0=OP.is_equal)
        a_t = sb.tile([128, 4, W], dt.bfloat16, tag="sa", name="a_t")
        b_t = sb.tile([128, 4, W], dt.bfloat16, tag="sb_", name="b_t")
        for fi in range(8):
            dst = a_t if fi < 4 else b_t
            fk = fi % 4
            pt = pc_tile()
            nc.tensor.matmul(pt[:], emb_sb[:, :, fi, :], oh[:, :, MAIN],
                             start=True, stop=True, perf_mode=DR)
            pth = pb_tile(HALO)
            nc.tensor.matmul(pth[:], emb_sb[:, :, fi, :], oh[:, :, 0:HALO],
                             start=True, stop=True, perf_mode=DR)
            if fi % 2 == 0:
                nc.scalar.mul(dst[:, fk, MAIN], pt[:], 1.0 / S_E)
            else:
                nc.vector.tensor_scalar(dst[:, fk, MAIN], pt[:], 1.0 / S_E,
                                        None, op0=OP.mult)
            nc.scalar.mul(dst[:, fk, 0:HALO], pth[:], 1.0 / S_E)

        bfp8 = sb.tile([128, 4, PITCH], dt.float8e4, tag="bf8", bufs=2,
                       name="bfp8")
        for u in range(2):
            nc.vector.tensor_scalar(bfp8[:, 2 * u:2 * u + 2, 0:W],
                                    b_t[:, 2 * u:2 * u + 2, 0:W], S_B, None,
                                    op0=OP.mult)

        # ---- layers ----
        c_tiles = [sb.tile([128, 4, W], dt.bfloat16, tag=f"scc{l}",
                           name=f"c_t{l}") for l in range(L)]
        y_t = sb.tile([128, 4, W], dt.bfloat16, tag="y", name="y_t")
        sc_t = sb.tile([128, 4, W], dt.bfloat16, tag="sct", name="sc_t")
        ysq = sb.tile([128, 4, W], dt.bfloat16, tag="ysq", name="ysq")
        srow_sb = sb.tile([1, W], dt.bfloat16, tag="srow", bufs=2, name="srow")
        grow = sb.tile([1, W], dt.bfloat16, tag="grow", bufs=2, name="grow")
        rt = sb.tile([1, W], dt.float32, tag="rt", bufs=2, name="rt")
        rt2 = sb.tile([1, W], dt.float32, tag="rt2", bufs=2, name="rt2")
        srt = sb.tile([1, W], dt.float32, tag="srt", bufs=2, name="srt")
        pmb = sb.tile([128, CHUNK], dt.bfloat16, tag="pmb", bufs=2, name="pmb")
        pgb = sb.tile([128, CHUNK], dt.bfloat16, tag="pgb", bufs=2, name="pgb")

        def get_w(l, j):
            if (l, j) in w_tiles:
                return w_tiles[(l, j)]
            w0t = sb.tile([128, 8, 4, 128], dt.float8e4, tag="w0",
                          bufs=3, name="w0t")
            nc.sync.dma_start(out=w0t[:], in_=d_w0[(l, j)][:])
            w1t = sb.tile([128, 8, 24, 128], dt.float8e4, tag="w1",
                          bufs=3, name="w1t")
            nc.sync.dma_start(out=w1t[:, 0:4], in_=d_w1[(l, j)][:, 0:4])
            nc.sync.dma_start(out=w1t[:, 4:8], in_=d_w1[(l, j)][:, 4:8])
            w2t = sb.tile([128, 4, 8, 128], dt.float8e4, tag="w2",
                          bufs=3, name="w2t")
            nc.sync.dma_start(out=w2t[:], in_=d_w2[(l, j)][:])
            w_tiles[(l, j)] = (w0t, w1t, w2t)
            return w_tiles[(l, j)]

        H1 = slice(HALO, HALO + 256)
        H2 = slice(HALO + 256, W)

        def emit_conv0(l, j, bfp8_t, split=False):
            """conv0 (1x1, F->I) + relu; bias = W0@fe (host precomputed).
            split=True processes the two column halves independently so work
            can begin before the full-width bfp8 is ready."""
            r0 = 2 * l
            hw0 = HALO - r0
            lj = 3 * l + j
            w0t = get_w(l, j)[0]
            cols = [(H1, 256), (H2, 256)] if split else [(MAIN, CHUNK)]
            x1 = sb.tile([128, 8, PITCH], dt.float8e4, tag="x1", bufs=2,
                         name=f"x1_{lj}")
            for ic in range(8):
                for sl, wdt in cols:
                    pt = pc_tile(wdt)
                    for u in range(2):
                        nc.tensor.matmul(pt[:], w0t[:, ic, 2 * u:2 * u + 2, :],
                                         bfp8_t[:, 2 * u:2 * u + 2, sl],
                                         start=(u == 0), stop=(u == 1),
                                         perf_mode=DR)
                    nc.scalar.activation(x1[:, ic, sl], pt[:], AF.Relu,
                                         bias=w0fe_sb[:, lj, ic:ic + 1],
                                         scale=1.0 / (S_W * S_B))
                pth = pb_tile(hw0)
                for u in range(2):
                    nc.tensor.matmul(pth[:], w0t[:, ic, 2 * u:2 * u + 2, :],
                                     bfp8_t[:, 2 * u:2 * u + 2, r0:HALO],
                                     start=(u == 0), stop=(u == 1),
                                     perf_mode=DR)
                nc.vector.tensor_scalar(x1[:, ic, r0:HALO], pth[:],
                                        1.0 / (S_W * S_B),
                                        w0feh_sb[:, lj, ic:ic + 1],
                                        op0=OP.mult, op1=OP.add)
                nc.vector.tensor_scalar(x1[:, ic, r0:HALO],
                                        x1[:, ic, r0:HALO], 0.0, None,
                                        op0=OP.max)
            return x1

        def emit_conv1(l, j, x1):
            """conv1 (k=3 causal, I->I) + relu."""
            r1 = 2 * l + 2
            hw1 = HALO - r1
            lj = 3 * l + j
            w1t = get_w(l, j)[1]
            x2 = sb.tile([128, 8, PITCH], dt.float8e4, tag="x2", bufs=2,
                         name=f"x2_{lj}")
            for oi in range(8):
                pt = pc_tile()
                for k in range(KW):
                    for u in range(4):
                        c = k * 8 + 2 * u
                        nc.tensor.matmul(
                            pt[:], w1t[:, oi, c:c + 2, :],
                            x1[:, 2 * u:2 * u + 2, 2 + k:2 + k + CHUNK],
                            start=(k == 0 and u == 0),
                            stop=(k == KW - 1 and u == 3), perf_mode=DR)
                if oi % 4 != 3:
                    nc.scalar.activation(x2[:, oi, MAIN], pt[:], AF.Relu,
                                         scale=1.0 / S_W)
                else:
                    nc.vector.tensor_scalar(x2[:, oi, MAIN], pt[:],
                                            1.0 / S_W, 0.0,
                                            op0=OP.mult, op1=OP.max)
                if hw1 > 0:
                    if oi == 0:
                        pth_all = ps.tile([128, 8, hw1], dt.float32,
                                          tag="pr", bufs=2, name="pth1")
                    for k in range(KW):
                        for u in range(4):
                            c = k * 8 + 2 * u
                            nc.tensor.matmul(
                                pth_all[:, oi, :], w1t[:, oi, c:c + 2, :],
                                x1[:, 2 * u:2 * u + 2,
                                   r1 - 2 + k:r1 - 2 + k + hw1],
                                start=(k == 0 and u == 0),
                                stop=(k == KW - 1 and u == 3),
                                perf_mode=DR)
                    if oi == 7:
                        nc.vector.tensor_scalar(x2[:, :, r1:HALO],
                                                pth_all[:], 1.0 / S_W, 0.0,
                                                op0=OP.mult, op1=OP.max)
            return x2

        def emit_conv2(l, j, x2):
            """conv2 (1x1, I->F). For j=0 the weights are host-side
            cumsum-fused (w2cum), so the PSUM holds cum directly and the
            evacuation multiplies by 1/(pos+1) (idb, with 1/S_W2C folded)."""
            r1 = 2 * l + 2
            hw1 = HALO - r1
            w2t = get_w(l, j)[2]
            for fc in range(4):
                pt = pc_tile()
                for u in range(4):
                    nc.tensor.matmul(pt[:], w2t[:, fc, 2 * u:2 * u + 2, :],
                                     x2[:, 2 * u:2 * u + 2, MAIN],
                                     start=(u == 0), stop=(u == 3),
                                     perf_mode=DR)
                if j == 0:
                    nc.vector.tensor_tensor(y_t[:, fc, MAIN], pt[:],
                                            idb[:, MAIN], op=OP.mult)
                elif j == 1:
                    nc.vector.tensor_scalar(sc_t[:, fc, MAIN], pt[:],
                                            1.0 / S_W, None, op0=OP.mult)
                else:
                    nc.vector.scalar_tensor_tensor(
                        y_t[:, fc, MAIN], pt[:], 1.0 / S_W,
                        y_t[:, fc, MAIN], op0=OP.mult, op1=OP.add)
                if hw1 > 0:
                    if fc == 0:
                        pth2 = ps.tile([128, 4, hw1], dt.float32,
                                       tag="pr", bufs=2, name="pth2")
                    for u in range(4):
                        nc.tensor.matmul(pth2[:, fc, :],
                                         w2t[:, fc, 2 * u:2 * u + 2, :],
                                         x2[:, 2 * u:2 * u + 2, r1:HALO],
                                         start=(u == 0), stop=(u == 3),
                                         perf_mode=DR)
                    if fc == 3:
                        if j == 0:
                            hb = bass.AP(tensor=idb.tensor, offset=idb[:, r1:HALO].offset,
                                         ap=[[idb[:].ap[0][0], 128], [0, 4],
                                             [1, hw1]])
                            nc.vector.tensor_tensor(y_t[:, :, r1:HALO],
                                                    pth2[:], hb, op=OP.mult)
                        elif j == 1:
                            nc.vector.tensor_scalar(sc_t[:, :, r1:HALO],
                                                    pth2[:], 1.0 / S_W, None,
                                                    op0=OP.mult)
                        else:
                            nc.vector.scalar_tensor_tensor(
                                y_t[:, :, r1:HALO], pth2[:], 1.0 / S_W,
                                y_t[:, :, r1:HALO], op0=OP.mult, op1=OP.add)

        for l in range(L):
            r0, r1 = 2 * l, 2 * l + 2
            hw0 = HALO - r0          # x1 halo width
            hw1 = HALO - r1          # conv1+ halo width (2 for l=0, 0 for l=1)

            # software pipeline: conv0 of FFN j+1 is emitted before conv1 of
            # FFN j so the PE has ready work during evacuation waits
            x1_next = emit_conv0(l, 0, bfp8, split=True)

            # coupling helpers, computed cheaply under the convs:
            # ptmp = beta*a (-> c = ptmp + t2), u = ptmp + b (-> b_new = u + t2)
            a_src = a_t if l == 0 else c_tiles[0]
            ptmp_t = sb.tile([128, 4, W], dt.bfloat16, tag="ptmp", bufs=2,
                             name="ptmp")
            nc.gpsimd.tensor_scalar(ptmp_t[:, :, r1:], a_src[:, :, r1:],
                                    float(np.float32(BETA)), None,
                                    op0=OP.mult)
            u_t = sb.tile([128, 4, W], dt.bfloat16, tag="u", bufs=2,
                          name="u_t")
            nc.gpsimd.tensor_tensor(u_t[:, :, r1:], ptmp_t[:, :, r1:],
                                    b_t[:, :, r1:], op=OP.add)

            for j in range(3):
                x1 = x1_next
                x2 = emit_conv1(l, j, x1)
                if j < 2:
                    x1_next = emit_conv0(l, j + 1, bfp8)
                emit_conv2(l, j, x2)

                if j == 1:
                    nc.gpsimd.tensor_tensor(y_t[:, :, r1:], y_t[:, :, r1:],
                                            sc_t[:, :, r1:], op=OP.mult)

            # ---- norm stats + momentum coupling ----
            # cell' = 256*(1-beta)*INIT_SCALE*(y-mu)*g enters as
            # t2 = y*G - M with G = 256*gs*g, M = G*mu rows broadcast.
            # The next layer's conv0 input is bfp8 = P + t2 with
            # P = 256*(b + beta*a) precomputed during the convs; the c/b
            # stream updates happen off the critical path (c = bfp8/256 - b).
            c_t = c_tiles[l]
            a_src = a_t if l == 0 else c_tiles[0]
            bfp8n = None
            if l == 0:
                bfp8n = sb.tile([128, 4, PITCH], dt.float8e4, tag="bf8",
                                bufs=2, name="bfp8n")
            for fq in range(4):
                nc.vector.tensor_tensor(ysq[:, fq, r1:], y_t[:, fq, r1:],
                                        y_t[:, fq, r1:], op=OP.mult)
            lpt = []
            if l == 1:
                # open the logits accumulation groups now; chunk matmuls are
                # interleaved into the coupling below
                for cc in range(2):
                    pt = pc_tile()
                    nc.tensor.matmul(pt[:], ob_sb[0:1, cc * 128:(cc + 1) * 128],
                                     ones_row[:1, :], start=True, stop=False)
                    lpt.append(pt)
            rngs = [(MAIN, CHUNK, 0)]
            if hw1 > 0:
                rngs.append((slice(r1, HALO), hw1, 0))
            for sl, wdt, off in rngs:
                prh1 = pr_tile(wdt)
                prh2 = pr_tile(wdt)
                pr1 = prh1[0:1, 0:wdt]
                pr2 = prh2[0:1, 0:wdt]
                for fk in range(4):
                    nc.tensor.matmul(pr1, ones_cb[:], y_t[:, fk, sl],
                                     start=(fk == 0), stop=(fk == 3))
                    nc.tensor.matmul(pr2, ones_cb[:], ysq[:, fk, sl],
                                     start=(fk == 0), stop=(fk == 3))
                # g = 1/sqrt((q - s^2/F)/F + 1e-9); Abs_reciprocal_sqrt
                # shares an act-func set with Relu/Copy/Square, so the
                # whole layer phase needs no table switches
                nc.vector.tensor_copy(srow_sb[:, sl], pr1)
                pm = ps.tile([128, wdt], dt.float32, tag="pc", bufs=6,
                             name="pm")
                nc.tensor.matmul(pm[:, :], bcmean[:], srow_sb[:1, sl],
                                 start=True, stop=True)
                nc.scalar.activation(rt[:, sl], pr1, AF.Square)
                nc.vector.scalar_tensor_tensor(rt2[:, sl], rt[:, sl],
                                               -1.0 / F, pr2,
                                               op0=OP.mult, op1=OP.add)
                nc.scalar.activation(grow[:, sl], rt2[:, sl],
                                     AF.Abs_reciprocal_sqrt,
                                     bias=eps_t[:], scale=1.0 / F)
                pg = ps.tile([128, wdt], dt.float32, tag="pc", bufs=6,
                             name="pg")
                nc.tensor.matmul(pg[:, :], bcg[:], grow[:1, sl],
                                 start=True, stop=True)
                nc.vector.tensor_copy(pmb[:, off:off + wdt], pm[:, :])
                nc.scalar.copy(pgb[:, off:off + wdt], pg[:, :])
                pmv, pgv = pmb[:, off:off + wdt], pgb[:, off:off + wdt]
                # t2 = (y-mu)*g*gs in-place in y; then c = ptmp+t2 and
                # b_new = u+t2 are single adds. fk3's t1/t2 run on Pool.
                for fk in range(4):
                    eng = nc.gpsimd if fk == 3 else nc.vector
                    eng.tensor_tensor(y_t[:, fk, sl], y_t[:, fk, sl],
                                      pmv, op=OP.subtract)
                    eng.tensor_tensor(y_t[:, fk, sl], y_t[:, fk, sl],
                                      pgv, op=OP.mult)
                    if l == 0:
                        nc.vector.tensor_tensor(b_t[:, fk, sl],
                                                u_t[:, fk, sl],
                                                y_t[:, fk, sl], op=OP.add)
                        nc.gpsimd.tensor_tensor(c_t[:, fk, sl],
                                                ptmp_t[:, fk, sl],
                                                y_t[:, fk, sl], op=OP.add)
                        if fk % 2 == 1:
                            nc.vector.tensor_scalar(
                                bfp8n[:, fk - 1:fk + 1, sl],
                                b_t[:, fk - 1:fk + 1, sl], S_B, None,
                                op0=OP.mult)
                    else:
                        nc.vector.tensor_tensor(c_t[:, fk, sl],
                                                ptmp_t[:, fk, sl],
                                                y_t[:, fk, sl], op=OP.add)
                        for cc in range(2):
                            nc.tensor.matmul(lpt[cc][:], ow_sb[:, fk, cc, :],
                                             c_t[:, fk, MAIN],
                                             start=False, stop=False)
                        nc.vector.tensor_tensor(b_t[:, fk, sl],
                                                u_t[:, fk, sl],
                                                y_t[:, fk, sl], op=OP.add)
                        for cc in range(2):
                            nc.tensor.matmul(lpt[cc][:],
                                             ow_sb[:, 4 + fk, cc, :],
                                             b_t[:, fk, MAIN],
                                             start=False,
                                             stop=(fk == 3))
            if l == 0:
                bfp8 = bfp8n
            else:
                # prefetch the Exp act table while coupling finishes; reading
                # grow pins it after this layer's Abs_reciprocal_sqrt
                nc.scalar.activation(scr[:], grow[0:1, 0:1], AF.Exp)

        # ---- final: log_softmax + NLL (logits accumulated above) ----
        otz = sb.tile([128, 2, CHUNK], dt.bfloat16, tag="otz", name="otz")
        expv = sb.tile([128, 2, CHUNK], dt.bfloat16, tag="expv", name="expv")
        for cc in range(2):
            nc.vector.tensor_tensor(otz[:, cc, :], oht[:, cc, :],
                                    lpt[cc][:], op=OP.mult)
            nc.scalar.activation(expv[:, cc, :], lpt[cc][:], AF.Exp)
        prl = pr_tile()
        prs = pr_tile()
        for cc in range(2):
            nc.tensor.matmul(prl[:1, :], ones_cb[:], otz[:, cc, :],
                             start=(cc == 0), stop=(cc == 1))
            nc.tensor.matmul(prs[:1, :], ones_cb[:], expv[:, cc, :],
                             start=(cc == 0), stop=(cc == 1))
        # ship sumexp (cols 0:512) and gathered logit (cols 512:1024);
        # host does the log
        outrow = sb.tile([1, 2 * CHUNK], dt.float32, tag="nll", name="outrow")
        nc.vector.tensor_copy(outrow[0:1, CHUNK:2 * CHUNK], prl[:1, :])
        nc.vector.tensor_copy(outrow[0:1, 0:CHUNK], prs[:1, :])
        nc.sync.dma_start(out=d_nll[:], in_=outrow[0:1, :])

    nc.compile()
    return nc


def _prep_host(inputs):
    """Host-side sharding/layout prep. Returns per-core input maps."""
    bf16 = ml_dtypes.bfloat16
    f8 = ml_dtypes.float8_e4m3
    inp = np.asarray(inputs["inp"])
    tgt = np.asarray(inputs["tgt"])
    emb = np.asarray(inputs["emb"], dtype=np.float32)
    w0s = np.asarray(inputs["w0s"], dtype=np.float32)
    w1s = np.asarray(inputs["w1s"], dtype=np.float32)
    w2s = np.asarray(inputs["w2s"], dtype=np.float32)
    out_w = np.asarray(inputs["out_w"], dtype=np.float32)
    out_b = np.asarray(inputs["out_b"], dtype=np.float32)

    shared = {}
    # emb8[p, ck, fi, col] = emb[ck*128+p, fi*128+col] * S_E
    shared["emb8"] = np.ascontiguousarray(
        (emb * S_E).reshape(2, 128, 8, 128).transpose(1, 0, 2, 3)).astype(f8)
    # owT[p, f2k, cc, col] = out_w[cc*128+col, f2k*128+p, 0]
    shared["owT"] = np.ascontiguousarray(
        out_w[:, :, 0].reshape(2, 128, 8, 128).transpose(3, 2, 0, 1)
    ).astype(bf16)
    shared["obrow"] = out_b.reshape(1, 256).astype(bf16)
    # feature embedding (fp32 math, matches reference _feature_embd)
    f = np.arange(F, dtype=np.float32)[:, None] + np.float32(1.0)
    additive = f % np.float32(2.0)
    f = (f - additive) / np.float32(2.0)
    f = f * np.float32(8.0 / F) - np.float32(math.log(C / (2.0 * math.pi)))
    fe = (np.exp(f) + additive * np.float32(math.pi))[:, 0]  # [F]
    w0fe = np.zeros((128, 6, 8), dtype=np.float32)
    for l in range(L):
        for j in range(3):
            lj = 3 * l + j
            v = (w0s[l, j, :, :, 0].astype(np.float64) @ fe.astype(np.float64)
                 ).astype(np.float32)  # [I]
            w0fe[:, lj, :] = v.reshape(8, 128).T
            # w0_[p, ic, c, col] = w0s[l,j, ic*128+col, c*128+p, 0] * S_W
            shared[f"w0_{l}{j}"] = np.ascontiguousarray(
                (w0s[l, j, :, :, 0] * S_W).reshape(8, 128, 4, 128)
                .transpose(3, 0, 2, 1)).astype(f8)
            # w1_[p, oi, k*8+ik, col] = w1s[l,j, oi*128+col, ik*128+p, k]*S_W
            shared[f"w1_{l}{j}"] = np.ascontiguousarray(
                (w1s[l, j] * S_W).reshape(8, 128, 8, 128, 3)
                .transpose(3, 0, 4, 2, 1).reshape(128, 8, 24, 128)).astype(f8)
            # w2_[p, fc, c, col] = w2s[l,j, fc*128+col, c*128+p, 0] * S_W
            # j=0: host-fused feature cumsum (prefix sums over f_out) * S_W2C
            if j == 0:
                w2m = np.clip(np.cumsum(w2s[l, j, :, :, 0], axis=0,
                                        dtype=np.float64).astype(np.float32)
                              * S_W2C, -240.0, 240.0)
            else:
                w2m = w2s[l, j, :, :, 0] * S_W
            shared[f"w2_{l}{j}"] = np.ascontiguousarray(
                w2m.reshape(4, 128, 8, 128).transpose(3, 0, 2, 1)).astype(f8)
    shared["w0fe"] = w0fe

    per_core = []
    for core in range(NCORES):
        b, q = core // 4, core % 4
        pos0 = q * CHUNK
        absidx = pos0 - HALO + np.arange(W)
        valid = absidx >= 0
        inprow = np.where(valid, inp[b, np.where(valid, absidx, 0)], PAD_IDX
                          ).astype(np.float32)
        idvrow = (np.where(valid, 1.0 / np.maximum(absidx + 1.0, 1.0), 1.0)
                  / S_W2C).astype(np.float32)
        tgtrow = tgt[b, pos0:pos0 + CHUNK].astype(np.float32)
        m = dict(shared)
        m["inprow"] = inprow
        m["tgtrow"] = tgtrow
        m["idvrow"] = idvrow
        m["w0feh"] = w0fe * np.float32(1.0 if q > 0 else 0.0)
        per_core.append(m)
    return per_core


def kernel(**inputs):
    if "nc" not in _CACHE:
        _CACHE["nc"] = _build()
    nc = _CACHE["nc"]
    in_maps = _prep_host(inputs)
    trace = TRACE
    if trace:
        try:
            from antenv.axon_hooks import get_axon_ntff_profile_hook  # noqa: F401
        except ImportError:
            trace = False
    res = bass_utils.run_bass_kernel_spmd(nc, in_maps, core_ids=list(range(NCORES)),
                                          trace=trace)
    if trace and res.exec_time_ns is not None:
        _CACHE["exec_time_ns"] = res.exec_time_ns
    nll = np.concatenate([np.log(r["nll"][0, :CHUNK].astype(np.float64))
                          - r["nll"][0, CHUNK:].astype(np.float64)
                          for r in res.results])
    return np.float32(nll.mean())
